# revision 8
# baseline (speedup 1.0000x reference)
"""Trainium2 Bass kernel for AttentionAggregationNN (ragged single-query MHA pooling).

Algebraic reduction: with one shared query vector, softmax-attention pooling per
group collapses to, per instance i and head h:
    e[i,h]   = exp(s_h . x_i)          (softmax shift-invariance drops the
                                        constant logit term)
    val[i,h] = t_h . x_i
    out[g]   = sum_h (sum_{i in g} e*val) / (sum_{i in g} e) + CONST
where s_h = Wk_h^T q_h / sqrt(D), t_h = Wv_h^T (w_lin @ w_out)_h, folded on the
host in float64.

Device work per core (data-parallel over groups, host pre-sorts by group):
  mm1: SP[128,16] = XT_subtile.T @ W16            (scores | vals), f32 PSUM
  ACT: e = exp(scores);  DVE: ev = e * vals       (batched over 8 subtiles)
  one-hot M[i,j] = (rel_gid[i] == j)              (one DVE op per batch,
                                                   stride-0 broadcast AP)
  mm2: acc[16, win] += [e|ev].T @ M               (segment sums into PSUM)
  epilogue per group-half: transpose, recip*mul, reduce, +CONST, DMA out.

Each core owns 256 contiguous groups, split into two halves of 128 groups; each
half's rows are padded to a multiple of 512 so one-hot windows never straddle
the half boundary and the first half's epilogue overlaps the second half's
compute.
"""
import os

if os.environ.get("AXON_H4_ENABLED") == "1" or os.environ.get("AXON_TERMINAL_JOB_NAME"):
    plats = os.environ.get("JAX_PLATFORMS", "")
    if "axon" not in plats:
        os.environ["JAX_PLATFORMS"] = "axon,cpu"

import numpy as np

# ---------------------------------------------------------------- problem dims
N, G, E, H, D = 131072, 2048, 256, 8, 32
NCORES = 8
GC = G // NCORES        # 256 groups per core
HC = GC // 2            # 128 groups per half
P = 128                 # partition dim / subtile rows
BATCH = 8               # subtiles per ACT/DVE batch
HALF_QUANT = 512        # row padding quantum per half (= P * min batch 4)

USE_BF16 = True

_CACHE: dict = {}


# ---------------------------------------------------------------- host algebra
def _fold_params(query, w_in, b_in, w_out, b_out, w_lin, b_lin):
    q64 = query.reshape(E).astype(np.float64)
    w64, b64 = w_in.astype(np.float64), b_in.astype(np.float64)
    wq, wk, wv = w64[:E], w64[E:2 * E], w64[2 * E:]
    bq, bk, bv = b64[:E], b64[E:2 * E], b64[2 * E:]
    q = wq @ q64 + bq
    qh = q.reshape(H, D)
    S = np.einsum("hde,hd->he", wk.reshape(H, D, E), qh) / np.sqrt(D)
    u = (w_lin.astype(np.float64) @ w_out.astype(np.float64)).reshape(E)
    uh = u.reshape(H, D)
    T = np.einsum("hde,hd->he", wv.reshape(H, D, E), uh)
    const = float(np.einsum("hd,hd->", uh, bv.reshape(H, D))
                  + w_lin.astype(np.float64).reshape(E) @ b_out.astype(np.float64)
                  + b_lin.astype(np.float64)[0])
    W16 = np.concatenate([S.T, T.T], axis=1)    # [E, 16]
    return W16, const


def _shard_prep(tree_preds, group_ids, np_dtype):
    sizes = np.bincount(group_ids, minlength=G)
    offsets = np.concatenate([[0], np.cumsum(sizes)]).astype(np.int64)
    sorter = np.argsort(group_ids, kind="stable")
    Xs = np.ascontiguousarray(tree_preds[sorter])
    gs = group_ids[sorter].astype(np.int64)

    # per (core, half): row range and size
    hstart = offsets[(np.arange(2 * NCORES) * HC)]
    hend = offsets[(np.arange(2 * NCORES) + 1) * HC]
    hrows = (hend - hstart).reshape(NCORES, 2)
    caps = [int(np.ceil(hrows[:, h].max() / HALF_QUANT) * HALF_QUANT) for h in (0, 1)]
    rows_cap = caps[0] + caps[1]
    nsub = rows_cap // P
    nsub_half = [caps[0] // P, caps[1] // P]

    # relative gid within half, -1 for pad rows
    grel = np.full((NCORES, rows_cap), -1, np.int64)
    for c in range(NCORES):
        for h in (0, 1):
            i = 2 * c + h
            n = hend[i] - hstart[i]
            col0 = h * caps[0]
            grel[c, col0:col0 + n] = gs[hstart[i]:hend[i]] - (c * GC + h * HC)
    gsub = grel.reshape(NCORES, nsub, P)
    lo = np.where(gsub >= 0, gsub, G).min(axis=(0, 2))
    hi = np.where(gsub >= 0, gsub, -1).max(axis=(0, 2))
    span = int(np.where(hi >= 0, hi - np.minimum(lo, hi) + 1, 1).max())
    win = 32 if span <= 28 else (64 if span <= 60 else 128)
    assert span <= win, f"one-hot span {span} > {win}"
    woff = np.minimum(np.where(lo < G, lo, 0), HC - win).astype(np.int64)
    assert ((hi < woff + win) | (hi < 0)).all()

    rel = np.where(gsub >= 0, gsub - woff[None, :, None], win).astype(np.float32)
    assert ((rel >= 0) & (rel <= win)).all()
    RELT = np.ascontiguousarray(rel.transpose(0, 2, 1)).astype(np_dtype)  # [NC,P,nsub]

    XT = np.zeros((NCORES, 2, P, rows_cap), np_dtype)
    for c in range(NCORES):
        for h in (0, 1):
            i = 2 * c + h
            n = hend[i] - hstart[i]
            col0 = h * caps[0]
            blk = Xs[hstart[i]:hend[i]].T.astype(np_dtype)
            XT[c, 0, :, col0:col0 + n] = blk[:P]
            XT[c, 1, :, col0:col0 + n] = blk[P:]
    return XT, RELT, woff, caps, nsub_half, win


def _macro_schedule(cap, first_half):
    """DMA macro-tile sizes covering `cap` rows; small tiles first on the very
    first half so compute starts early, 4096 steady-state."""
    sizes = []
    ramp = [512, 1024, 2048] if first_half else [2048]
    left = cap
    for r in ramp:
        if left <= 0:
            break
        t = min(r, left)
        sizes.append(t)
        left -= t
    while left > 0:
        t = min(4096, left)
        sizes.append(t)
        left -= t
    assert sum(sizes) == cap and all(s % HALF_QUANT == 0 for s in sizes)
    return sizes


# ---------------------------------------------------------------- bass program
def _build_program(caps, nsub_half, woff, const, win):
    import concourse.bass as bass
    import concourse.tile as tile
    from concourse import bacc, mybir
    from concourse.masks import make_identity

    DT = mybir.dt.bfloat16 if USE_BF16 else mybir.dt.float32
    F32 = mybir.dt.float32
    Exp = mybir.ActivationFunctionType.Exp
    Alu = mybir.AluOpType
    rows_cap = caps[0] + caps[1]
    nsub = nsub_half[0] + nsub_half[1]
    JW = BATCH * win

    nc = bacc.Bacc(None, target_bir_lowering=False)
    xt = nc.dram_tensor("xt", [2, P, rows_cap], DT, kind="ExternalInput")
    relt = nc.dram_tensor("relt", [P, nsub], DT, kind="ExternalInput")
    wmat = nc.dram_tensor("wmat", [2, P, 16], DT, kind="ExternalInput")
    jiota = nc.dram_tensor("jiota", [P, JW], DT, kind="ExternalInput")
    out = nc.dram_tensor("out", [GC], F32, kind="ExternalOutput")

    with tile.TileContext(nc) as tc:
        with (
            tc.tile_pool(name="const", bufs=1) as constp,
            tc.tile_pool(name="xtp", bufs=2) as xtp,
            tc.tile_pool(name="work", bufs=3) as workp,
            tc.tile_pool(name="ep", bufs=1) as epsb,
            tc.tile_pool(name="mm1", bufs=4, space="PSUM") as mm1p,
            tc.tile_pool(name="acc", bufs=1, space="PSUM") as accp,
            tc.tile_pool(name="tps", bufs=1, space="PSUM") as tpsp,
        ):
            # ---- constants
            w_t = constp.tile([P, 32], DT)
            nc.sync.dma_start(w_t[:, 0:16], wmat[0])
            nc.sync.dma_start(w_t[:, 16:32], wmat[1])
            j_t = constp.tile([P, JW], DT)
            nc.sync.dma_start(j_t[:], jiota[:])
            relt_t = constp.tile([P, nsub], DT)
            nc.sync.dma_start(relt_t[:], relt[:])
            zbias = constp.tile([P, 1], F32)
            nc.gpsimd.memset(zbias[:], 0.0)
            zw = constp.tile([P, 16], DT)
            nc.gpsimd.memset(zw[:], 0.0)
            ident = constp.tile([16, 16], F32)
            make_identity(nc, ident[:])

            accs = [accp.tile([16, HC], F32, tag=f"acc{h}", name=f"acc{h}")
                    for h in (0, 1)]
            for acc in accs:
                nc.tensor.matmul(acc[:, 0:HC], lhsT=zw[:], rhs=j_t[:, 0:HC],
                                 start=True, stop=False, skip_group_check=True)

            def epilogue(h):
                acc = accs[h]
                cc = epsb.tile([16, HC], F32, tag=f"cc{h}")
                nc.vector.tensor_copy(cc[:], acc[:])
                tp = tpsp.tile([P, 16], F32, tag=f"tp{h}")
                nc.tensor.transpose(tp[:], cc[:], ident[:])
                dd = epsb.tile([P, 16], F32, tag=f"dd{h}")
                nc.vector.tensor_copy(dd[:], tp[:])
                rec = epsb.tile([P, 8], F32, tag=f"rec{h}")
                nc.vector.reciprocal(rec[:], dd[:, 0:8])
                rr = epsb.tile([P, 8], F32, tag=f"rr{h}")
                nc.vector.tensor_tensor(rr[:], rec[:], dd[:, 8:16], op=Alu.mult)
                oo = epsb.tile([P, 1], F32, tag=f"oo{h}")
                nc.vector.tensor_reduce(oo[:], rr[:], axis=mybir.AxisListType.X,
                                        op=Alu.add)
                oo2 = epsb.tile([P, 1], F32, tag=f"oo2{h}")
                nc.vector.tensor_scalar_add(oo2[:], oo[:], float(const))
                nc.sync.dma_start(out[h * HC:(h + 1) * HC, None], oo2[:])

            # ---- main loop
            for h in (0, 1):
                col0 = 0 if h == 0 else caps[0]
                s = nsub_half[0] if h else 0
                last_s = (nsub_half[0] + nsub_half[1] if h else nsub_half[0]) - 1
                acc = accs[h]
                m0 = col0
                for msz in _macro_schedule(caps[h], first_half=(h == 0)):
                    x0 = xtp.tile([P, 4096], DT, tag="x0")
                    x1 = xtp.tile([P, 4096], DT, tag="x1")
                    nc.sync.dma_start(x0[:, 0:msz], xt[0, :, m0:m0 + msz])
                    nc.sync.dma_start(x1[:, 0:msz], xt[1, :, m0:m0 + msz])
                    b0 = 0
                    while b0 < msz:
                        bsz = min(BATCH, (msz - b0) // P)      # subtiles in batch
                        spp = mm1p.tile([P, 16 * BATCH], F32)
                        m_t = workp.tile([P, BATCH * win], DT, tag="m")
                        sp2 = workp.tile([P, 16 * BATCH], DT, tag="sp2")
                        for j in range(bsz):
                            col = b0 + j * P
                            nc.tensor.matmul(spp[:, j * 16:j * 16 + 16],
                                             lhsT=x0[:, col:col + P],
                                             rhs=w_t[:, 0:16],
                                             start=True, stop=False)
                            nc.tensor.matmul(spp[:, j * 16:j * 16 + 16],
                                             lhsT=x1[:, col:col + P],
                                             rhs=w_t[:, 16:32],
                                             start=False, stop=True)
                        # batched one-hot: M[i, b, w] = (rel[i, s+b] == w)
                        mv = m_t[:].rearrange("p (b w) -> p b w", w=win)
                        jv = j_t[:, 0:bsz * win].rearrange("p (b w) -> p b w", w=win)
                        relb = relt_t[:, s:s + bsz].to_broadcast([P, bsz, win])
                        nc.vector.tensor_tensor(mv[:, 0:bsz, :], jv, relb,
                                                op=Alu.is_equal)
                        spv = spp[:].rearrange("p (b c) -> p b c", c=16)
                        sp2v = sp2[:].rearrange("p (b c) -> p b c", c=16)
                        nc.scalar.activation(sp2v[:, 0:bsz, 0:8], spv[:, 0:bsz, 0:8],
                                             Exp, bias=zbias[:])
                        nc.vector.tensor_tensor(sp2v[:, 0:bsz, 8:16],
                                                sp2v[:, 0:bsz, 0:8],
                                                spv[:, 0:bsz, 8:16], op=Alu.mult)
                        for j in range(bsz):
                            nc.tensor.matmul(
                                acc[:, woff[s + j]:woff[s + j] + win],
                                lhsT=sp2[:, j * 16:j * 16 + 16],
                                rhs=m_t[:, j * win:(j + 1) * win],
                                start=False, stop=(s + j == last_s),
                                skip_group_check=True)
                        s += bsz
                        b0 += bsz * P
                    m0 += msz
                epilogue(h)
    nc.compile()
    return nc


# ---------------------------------------------------------------- entry point
def _invoke(tree_preds, group_ids, query, w_in, b_in, w_out, b_out, w_lin, b_lin,
            trace=False, **spmd_kwargs):
    import ml_dtypes
    np_dt = ml_dtypes.bfloat16 if USE_BF16 else np.float32

    tree_preds = np.asarray(tree_preds, dtype=np.float32)
    group_ids = np.asarray(group_ids, dtype=np.int32)

    W16, const = _fold_params(np.asarray(query), np.asarray(w_in), np.asarray(b_in),
                              np.asarray(w_out), np.asarray(b_out),
                              np.asarray(w_lin), np.asarray(b_lin))
    XT, RELT, woff, caps, nsub_half, win = _shard_prep(tree_preds, group_ids, np_dt)

    key = (tuple(caps), tuple(nsub_half), tuple(woff.tolist()), float(const), win)
    if _CACHE.get("key") != key:
        _CACHE["nc"] = _build_program(caps, nsub_half, woff, const, win)
        _CACHE["key"] = key
    nc = _CACHE["nc"]

    wmat = np.ascontiguousarray(W16.astype(np_dt).reshape(2, P, 16))
    jio = np.broadcast_to(np.arange(BATCH * win, dtype=np.float32) % win,
                          (P, BATCH * win)).astype(np_dt)
    jio = np.ascontiguousarray(jio)

    in_maps = [{"xt": XT[c], "relt": RELT[c], "wmat": wmat, "jiota": jio}
               for c in range(NCORES)]

    from concourse.bass_utils import run_bass_kernel_spmd
    res = run_bass_kernel_spmd(nc, in_maps, core_ids=list(range(NCORES)),
                               trace=trace, **spmd_kwargs)

    out = np.empty((G, 1), np.float32)
    for c in range(NCORES):
        out[c * GC:(c + 1) * GC, 0] = res.results[c]["out"]
    return out, res


def kernel(tree_preds, group_ids, query, w_in, b_in, w_out, b_out, w_lin, b_lin):
    out, _ = _invoke(tree_preds, group_ids, query, w_in, b_in,
                     w_out, b_out, w_lin, b_lin)
    return out
